# revision 14
# baseline (speedup 1.0000x reference)
"""GAT layer kernel for Trainium2, 8 NeuronCores, data-parallel.

Problem: nn_GATLayer (B=4, N=2048, F_IN=64, F_OUT=64, H=4).

Sharding: core c handles batch b = c//2 and destination-node rows
[ (c%2)*1024, (c%2)*1024+1024 ) of that batch (all heads, all source
nodes).  Every adjacency row is read exactly once across the 8 cores.

Per-core algorithm (transposed-score layout, j on partitions):
  h      = x @ W                       (PE, fp32 -> bf16)
  u_i    = h[i] . a_src[head],  v_j = h[j] . a_dst[head]
  w      = u_i + v_j + 200*(adj_ji - 1)          [j, i] tiles
  lrelu  = max(w, 0.2*w)     (exact LeakyReLU; masked entries end up
                              <= 0.2*s - 40 so exp() vanishes ~ 4e-18)
  e      = exp(lrelu)                  (ACT, single pass)
  num/den: PSUM accumulation of  [h_aug | 1]^T . e  over j-chunks
  out    = num / den                   (transpose back, row scale)
"""

import sys

sys.path.insert(0, "/opt/trn_rl_repo")

from contextlib import ExitStack

import numpy as np

import concourse.bass as bass
import concourse.mybir as mybir
import concourse.tile as tile
from concourse import bacc
from concourse.bass_utils import run_bass_kernel_spmd
from concourse.masks import make_identity

F32 = mybir.dt.float32
BF16 = mybir.dt.bfloat16
I32 = mybir.dt.int32
ALU = mybir.AluOpType
ACTF = mybir.ActivationFunctionType

B, N, F_IN, F_OUT, H = 4, 2048, 64, 64, 4
NI = N // 2            # destination rows per core
P = 128                # partitions
NJC = N // P           # 16 j-chunks
NIT = NI // P          # 8 i-tiles (per-core rows / 128)
NCC = 8                # adjacency column-chunks
CCW = N // NCC         # 256 columns per chunk
MASK_C = 200.0         # additive mask magnitude (0.2*200 = 40 => exp ~ 4e-18)


def gat_core_program(tc, outs, ins):
    """Build the per-core program.  ins/outs are dicts of DRAM APs.

    ins:  x [N, F_IN] f32 (full batch-b node features)
          xi [NI, F_IN] f32 (this core's destination rows of x)
          adj [NI, N] i32 (this core's destination rows of adjacency)
          w  [F_IN, H*F_OUT] f32
          attn [H, 2*F_OUT] f32
    outs: out [NI, H*F_OUT] f32
    """
    nc = tc.nc
    ctx = ExitStack()
    x_d, xi_d, adj_d, w_d, attn_d = (
        ins["x"], ins["xi"], ins["adj"], ins["w"], ins["attn"])
    out_d = outs["out"]
    HO = H * F_OUT  # 256

    const = ctx.enter_context(tc.tile_pool(name="const", bufs=1))
    sctx = ExitStack()
    setup_sb = sctx.enter_context(tc.tile_pool(name="setup_sb", bufs=2))
    setup_ps = sctx.enter_context(tc.tile_pool(name="setup_ps", bufs=2, space="PSUM"))

    # ---------------- persistent tensors ----------------
    ident = const.tile([P, P], F32)
    make_identity(nc, ident[:])

    m200T = const.tile([P, NJC, NI], BF16)        # 32KB/part
    haug = const.tile([P, NJC, H, F_OUT + 1], BF16)
    ubc = const.tile([P, H, NI], BF16)            # u broadcast across partitions
    vsc = const.tile([P, NJC, H], F32)            # v (j on partitions)
    v02 = const.tile([P, NJC, H], F32)            # 0.2*v
    outf = const.tile([P, NIT, HO], F32)          # final output staging

    # ---------------- setup: x^T, xi^T, W, attention ----------------
    w_sb = const.tile([F_IN, HO], F32)
    nc.sync.dma_start(w_sb[:], w_d[:])
    attn_sb = setup_sb.tile([H, 2 * F_OUT], F32)
    nc.sync.dma_start(attn_sb[:], attn_d[:])

    # x blocked [128, 16, 64]; xi blocked [128, 8, 64]
    x_sb = setup_sb.tile([P, NJC, F_IN], F32, tag="xload")
    nc.sync.dma_start(x_sb[:], x_d.rearrange("(s p) c -> p s c", p=P))
    xi_sb = setup_sb.tile([P, NIT, F_IN], F32, tag="xload")
    nc.sync.dma_start(xi_sb[:], xi_d.rearrange("(s p) c -> p s c", p=P))

    xT = const.tile([F_IN, N], F32)    # x^T
    xiT = const.tile([F_IN, NI], F32)  # xi^T
    for s in range(NJC):
        pt = setup_ps.tile([F_IN, P], F32, tag="sps")
        nc.tensor.transpose(pt[:], x_sb[:, s, :], ident[:])
        nc.vector.tensor_copy(xT[:, s * P:(s + 1) * P], pt[:])
    for s in range(NIT):
        pt = setup_ps.tile([F_IN, P], F32, tag="sps")
        nc.tensor.transpose(pt[:], xi_sb[:, s, :], ident[:])
        nc.vector.tensor_copy(xiT[:, s * P:(s + 1) * P], pt[:])

    # attn^T [128, 4]: rows 0:64 = a_src[h], 64:128 = a_dst[h]
    attnT = setup_sb.tile([P, H], F32)
    pt = setup_ps.tile([P, H], F32, tag="sps")
    nc.tensor.transpose(pt[:], attn_sb[:], ident[:H, :H])
    nc.vector.tensor_copy(attnT[:], pt[:])

    # W^T [128, 2, 64]
    wT = setup_sb.tile([P, 2, F_IN], F32)
    for half in range(2):
        pt = setup_ps.tile([P, F_IN], F32, tag="sps")
        nc.tensor.transpose(pt[:], w_sb[:, half * P:(half + 1) * P],
                            ident[:F_IN, :F_IN])
        nc.vector.tensor_copy(wT[:, half, :], pt[:])

    # AA [256, 8] stored [128, 2, 8]: AA[h*64+f, h] = a_src[h, f];
    # AA[h*64+f, 4+h] = a_dst[h, f].  Filled by partition-shifting S2S DMAs.
    aa = setup_sb.tile([P, 2, 2 * H], F32)
    nc.gpsimd.memset(aa[:], 0.0)
    for h in range(H):
        half, poff = divmod(h * F_OUT, P)
        nc.sync.dma_start(
            aa[poff:poff + F_OUT, half, h], attnT[0:F_OUT, h])
        nc.sync.dma_start(
            aa[poff:poff + F_OUT, half, H + h], attnT[F_OUT:2 * F_OUT, h])

    # WA [64, 8] = W @ AA
    wa = const.tile([F_IN, 2 * H], F32)
    pwa = setup_ps.tile([F_IN, 2 * H], F32, tag="sps")
    for half in range(2):
        nc.tensor.matmul(pwa[:], wT[:, half, :], aa[:, half, :],
                         start=(half == 0), stop=(half == 1))
    nc.vector.tensor_copy(wa[:], pwa[:])

    # uvT over full x: rows 0:4 -> u-heads, 4:8 -> v-heads.  [8, N]
    uvT = setup_sb.tile([2 * H, N], F32)
    for ch in range(N // 512):
        pv = setup_ps.tile([2 * H, 512], F32, tag="sps")
        nc.tensor.matmul(pv[:], wa[:], xT[:, ch * 512:(ch + 1) * 512],
                         start=True, stop=True)
        nc.vector.tensor_copy(uvT[:, ch * 512:(ch + 1) * 512], pv[:])
    # u rows over xi, one [1, NI] tile per head (base partition 0)
    u_rows = [const.tile([1, NI], F32, tag=f"urow{h}", name=f"urow{h}")
              for h in range(H)]
    for h in range(H):
        for ch in range(NI // 512):
            pv = setup_ps.tile([1, 512], F32, tag="sps")
            nc.tensor.matmul(pv[:], wa[:, h:h + 1],
                             xiT[:, ch * 512:(ch + 1) * 512],
                             start=True, stop=True)
            nc.vector.tensor_copy(u_rows[h][:, ch * 512:(ch + 1) * 512], pv[:])

    # v with j on partitions: transpose uvT 128-col blocks -> [128, 16, 8]
    for jc in range(NJC):
        pv = setup_ps.tile([P, 2 * H], F32, tag="sps")
        nc.tensor.transpose(pv[:], uvT[:, jc * P:(jc + 1) * P],
                            ident[:2 * H, :2 * H])
        nc.vector.tensor_copy(vsc[:, jc, :], pv[:, H:2 * H])
    nc.vector.tensor_scalar_mul(v02[:], vsc[:], 0.2)

    # u broadcast across partitions: ones[1,128]^T . uT[h] -> [128, NI] bf16
    ones_row = setup_sb.tile([1, P], F32)
    nc.gpsimd.memset(ones_row[:], 1.0)
    for h in range(H):
        pb = setup_ps.tile([P, NI], F32, tag="sps")
        for ch in range(NI // 512):
            nc.tensor.matmul(pb[:, ch * 512:(ch + 1) * 512], ones_row[:],
                             u_rows[h][:, ch * 512:(ch + 1) * 512],
                             start=True, stop=True)
        nc.vector.tensor_copy(ubc[:, h, :], pb[:])

    # ---------------- h = x @ W, build h_aug stationaries ----------------
    nc.gpsimd.memset(haug[:, :, :, F_OUT], 1.0)
    for s in range(NJC):
        ph = setup_ps.tile([P, HO], F32, tag="sps")
        nc.tensor.matmul(ph[:], xT[:, s * P:(s + 1) * P], w_sb[:],
                         start=True, stop=True)
        nc.vector.tensor_copy(
            haug[:, s, :, 0:F_OUT],
            ph.rearrange("p (h f) -> p h f", h=H))

    sctx.close()

    # ---------------- adjacency: load, convert, transpose ----------------
    adj_pool = ctx.enter_context(tc.tile_pool(name="adj", bufs=2))
    m200_pool = ctx.enter_context(tc.tile_pool(name="m200", bufs=2))
    for cc in range(NCC):
        a_i32 = adj_pool.tile([P, NIT, CCW], I32)
        nc.sync.dma_start(
            a_i32[:],
            adj_d.rearrange("(s p) c -> p s c", p=P)[:, :, cc * CCW:(cc + 1) * CCW])
        m_nat = m200_pool.tile([P, NIT, CCW], BF16)
        nc.gpsimd.tensor_scalar(m_nat[:], a_i32[:], MASK_C, -MASK_C,
                                op0=ALU.mult, op1=ALU.add)
        for sub in range(NIT):
            for half in range(CCW // P):
                jc = (cc * CCW + half * P) // P
                nc.sync.dma_start_transpose(
                    m200T[:, jc, sub * P:(sub + 1) * P],
                    m_nat[:, sub, half * P:(half + 1) * P])

    # ---------------- main: scores -> exp -> matmul ----------------
    bpool = ctx.enter_context(tc.tile_pool(name="bwork", bufs=3))
    epool = ctx.enter_context(tc.tile_pool(name="ework", bufs=3))
    po_pool = ctx.enter_context(tc.tile_pool(name="po", bufs=2, space="PSUM"))
    pt_pool = ctx.enter_context(tc.tile_pool(name="ptrans", bufs=2, space="PSUM"))
    ot_pool = ctx.enter_context(tc.tile_pool(name="otsb", bufs=2))
    rec_pool = ctx.enter_context(tc.tile_pool(name="rec", bufs=2))

    for h in range(H):
        po = po_pool.tile([F_OUT + 1, NI], F32)
        for jc in range(NJC):
            t0 = bpool.tile([P, NI], BF16, tag="t0")
            nc.vector.tensor_add(t0[:], m200T[:, jc, :], ubc[:, h, :])
            a_t = bpool.tile([P, NI], BF16, tag="a")
            nc.vector.tensor_scalar(a_t[:], t0[:], vsc[:, jc, h:h + 1], None,
                                    op0=ALU.add)
            b_t = bpool.tile([P, NI], BF16, tag="b")
            nc.vector.tensor_scalar(b_t[:], t0[:], 0.2, v02[:, jc, h:h + 1],
                                    op0=ALU.mult, op1=ALU.add)
            c_t = bpool.tile([P, NI], BF16, tag="c")
            nc.vector.tensor_tensor(c_t[:], a_t[:], b_t[:], op=ALU.max)
            e_t = epool.tile([P, NI], BF16, tag="e")
            nc.scalar.activation(e_t[:], c_t[:], ACTF.Exp)
            for mh in range(NI // 512):
                nc.tensor.matmul(po[:, mh * 512:(mh + 1) * 512],
                                 haug[:, jc, h, :],
                                 e_t[:, mh * 512:(mh + 1) * 512],
                                 start=(jc == 0), stop=(jc == NJC - 1))

        # epilogue for this head: transpose back, normalize
        import os
        emode = int(os.environ.get("GAT_EPIL_MODE", "3"))
        ot = ot_pool.tile([F_OUT + 1, NI], F32)
        if emode >= 1:
            nc.vector.tensor_copy(ot[:], po[:])
        for it in range(NIT):
            if emode < 2:
                break
            ptp = pt_pool.tile([P, F_OUT + 1], F32)
            nc.tensor.transpose(ptp[:], ot[:, it * P:(it + 1) * P],
                                ident[:F_OUT + 1, :F_OUT + 1])
            if emode < 3:
                continue
            rec = rec_pool.tile([P, 1], F32)
            nc.vector.reciprocal(rec[:], ptp[:, F_OUT:F_OUT + 1])
            nc.vector.tensor_scalar(
                outf[:, it, h * F_OUT:(h + 1) * F_OUT],
                ptp[:, 0:F_OUT], rec[:], None, op0=ALU.mult)
    if emode < 3:
        nc.gpsimd.memset(outf[:], 0.0)

    for it in range(NIT):
        nc.sync.dma_start(
            out_d.rearrange("(s p) c -> p s c", p=P)[:, it, :],
            outf[:, it, :])
    ctx.close()


N_CORES = 8
_CACHE = {}


def _build():
    if "nc" not in _CACHE:
        nc = bacc.Bacc("TRN2", target_bir_lowering=False, debug=False,
                       num_devices=N_CORES)
        ins = {
            "x": nc.dram_tensor("x", [N, F_IN], F32, kind="ExternalInput").ap(),
            "xi": nc.dram_tensor("xi", [NI, F_IN], F32, kind="ExternalInput").ap(),
            "adj": nc.dram_tensor("adj", [NI, N], I32, kind="ExternalInput").ap(),
            "w": nc.dram_tensor("w", [F_IN, H * F_OUT], F32,
                                kind="ExternalInput").ap(),
            "attn": nc.dram_tensor("attn", [H, 2 * F_OUT], F32,
                                   kind="ExternalInput").ap(),
        }
        outs = {"out": nc.dram_tensor("out", [NI, H * F_OUT], F32,
                                      kind="ExternalOutput").ap()}
        with tile.TileContext(nc) as tc:
            gat_core_program(tc, outs, ins)
        nc.compile()
        _CACHE["nc"] = nc
    return _CACHE["nc"]


def make_in_maps(node_features, adj_matrix, W, attention):
    node_features = np.ascontiguousarray(node_features, dtype=np.float32)
    adj_matrix = np.ascontiguousarray(adj_matrix, dtype=np.int32)
    W = np.ascontiguousarray(W, dtype=np.float32)
    attention = np.ascontiguousarray(attention, dtype=np.float32)
    in_maps = []
    for c in range(N_CORES):
        b, ih = divmod(c, 2)
        i0 = ih * NI
        in_maps.append({
            "x": node_features[b],
            "xi": np.ascontiguousarray(node_features[b, i0:i0 + NI]),
            "adj": np.ascontiguousarray(adj_matrix[b, i0:i0 + NI]),
            "w": W,
            "attn": attention,
        })
    return in_maps


def assemble(results):
    out = np.empty((B, N, H * F_OUT), dtype=np.float32)
    for c in range(N_CORES):
        b, ih = divmod(c, 2)
        i0 = ih * NI
        out[b, i0:i0 + NI] = results[c]["out"]
    return out


def kernel(node_features, adj_matrix, W, attention):
    nc = _build()
    in_maps = make_in_maps(node_features, adj_matrix, W, attention)
    res = run_bass_kernel_spmd(nc, in_maps, core_ids=list(range(N_CORES)))
    return assemble(res.results)
